# revision 40
# baseline (speedup 1.0000x reference)
"""Trainium2 8-core tensor-parallel attention kernel (Bass/Tile).

Problem: B=1, S=2048, D=4096, H=32 q-heads, KVH=8 kv-heads, HD=128, causal,
RoPE, GQA.  Sharding: tensor-parallel over heads — each of the 8 cores owns
4 q heads + 1 kv head (wq/wk/wv column shards), computes its heads' attention
output transposed [hd, sq], multiplies by its own wo ROW shard inline per
s-tile to produce a full-width partial [512, 4096], and ReduceScatters each
s-tile partial (add, bf16) across the 8 cores straight into the ExternalOutput.
Host reassembles: full[512*T + 64*c + i] = out_c[64*T + i].

Versus the AllGather formulation this keeps every matmul dependent only on
LOCAL data — the collective is pure background traffic and only the last
s-tile's ReduceScatter (~0.5 MB out) is exposed at the tail.

Compute dtype: bf16 operands / f32 accumulation.
Host-prepared layouts (every DMA a contiguous partition-major image):
  xtr  [4, 128, 32*512]  x^T tiles: xtr[T][p, 512*b+f] = x[512*T+f, 128*b+p]
  wqkv [128, 32*768]     [wq_perm | wk_perm | wv] d-block-major stationary tiles
  woc  [128, 32*512]     wo ROW shard: woc[p, h*4096+d] = wo[c*512+h*128+p, d]
  cst/snt [64, 2048]     RoPE tables transposed (bf16)
  m01  [128, 4*512]      0/1 causal masks for the 4 diagonal block offsets
RoPE trick: wq/wk columns are permuted per head (even channels first) so the
rotation acts on partition halves [0:64]/[64:128] of the projected qT/kT;
cos/sin tables are partition-doubled ([c;c], [s;s]) so RoPE is 4 DVE ops
instead of 6.  Softmax in the transposed score layout (scoresT[sk, sq])
without max-subtraction (|scores| <~ 15).  QK->exp->PV is software-pipelined
2 blocks deep so the Act-engine exp latency never stalls PE.  The denominator
is a per-block ones-vector matmul accumulated in a [1,512] PSUM bank (PE is
cheaper per column than DVE, and DVE was the attention bottleneck);
normalization via gpsimd partition-broadcast of the reciprocal row.
Pipeline order per tile: proj(T) -> attn(T) -> wo(T-1) -> RS(T-1), so the PE
queue never head-of-line blocks on tile T's own normalization chain.
"""

import math

import numpy as np
import ml_dtypes

import concourse.bass as bass
import concourse.bass_isa as bass_isa
import concourse.bacc as bacc
import concourse.tile as tile
import concourse.mybir as mybir
from concourse.bass_utils import run_bass_kernel_spmd

BF16 = mybir.dt.bfloat16
F32 = mybir.dt.float32
NPBF = ml_dtypes.bfloat16

N_CORES = 8
S, D, H, KVH, HD = 2048, 4096, 32, 8, 128
QH = H // N_CORES          # 4 q heads per core
NB = D // 128              # 32 contraction blocks
ST = S // 512              # 4 s-tiles
SCALE = 1.0 / math.sqrt(HD)

_CACHE = {}


def build_kernel(n_cores=N_CORES, with_collective=True, reps=1,
                 skip_final=False, ag_splits=4,
                 denom="dve", look=2, rope4=True):
    nc = bacc.Bacc("TRN2", target_bir_lowering=False, debug=False,
                   num_devices=n_cores)

    xtr = nc.declare_dram_parameter("xtr", [ST, 128, NB * 512], BF16, isOutput=False)
    wqkv = nc.declare_dram_parameter("wqkv", [128, NB * 768], BF16, isOutput=False)
    woc = nc.declare_dram_parameter("woc", [128, NB * 512], BF16, isOutput=False)
    cst = nc.declare_dram_parameter("cst", [128, S], BF16, isOutput=False)
    snt = nc.declare_dram_parameter("snt", [128, S], BF16, isOutput=False)
    m01 = nc.declare_dram_parameter("m01", [128, 4 * 512], BF16, isOutput=False)
    ident = nc.declare_dram_parameter("ident", [128, 128], BF16, isOutput=False)
    # RS output: chunk T -> rows [64*T, 64*(T+1))
    out = nc.declare_dram_parameter("out", [ST * 64, D], BF16, isOutput=True)

    with tile.TileContext(nc) as tc:
        with (
            tc.tile_pool(name="const", bufs=1) as constp,
            tc.tile_pool(name="dram", bufs=1, space="DRAM") as dram,
        ):
            # persistent SBUF images, chunked so the first matmuls start after
            # ~0.3 MB of DMA (Tile tracks deps per tile, not per slice)
            WQ_CHUNKS = ((0, 2), (2, 8), (8, 16), (16, 24), (24, 32))
            wqkv_sb = []
            for ci, (lo, hi) in enumerate(WQ_CHUNKS):
                t = constp.tile([128, 768 * (hi - lo)], BF16, name=f"wq_ch{ci}")
                nc.sync.dma_start(t[:], wqkv[:, 768 * lo: 768 * hi])
                wqkv_sb.append(t)
                if ci == 0:
                    cst_sb = constp.tile([128, S], BF16)
                    nc.gpsimd.dma_start(cst_sb[:], cst[:])
                    snt_sb = constp.tile([128, S], BF16)
                    nc.gpsimd.dma_start(snt_sb[:], snt[:])
                    m01_sb = constp.tile([128, 4 * 512], BF16)
                    nc.gpsimd.dma_start(m01_sb[:], m01[:])
                    ident_sb = constp.tile([128, 128], BF16)
                    nc.gpsimd.dma_start(ident_sb[:], ident[:])
                    ones_sb = constp.tile([128, 8], BF16)
                    nc.gpsimd.memset(ones_sb[:], 1.0)
            # wo row-shard halves: tiles allocated here, but the DMAs are
            # emitted inside the body AFTER the T=1 x-prefetch so the startup
            # window isn't contended (first wo use is ~130us in)
            woc_h = [constp.tile([128, 16 * 512], BF16, name=f"woc_h{ch}")
                     for ch in range(2)]

            for rep in range(reps):
                cc_in = [dram.tile([512, D], BF16, name=f"cc_in{rep}_{t}")
                         for t in range(ST)]
                cc_out = [dram.tile([64, D], BF16, name=f"cc_out{rep}_{t}")
                          for t in range(ST)]
                _emit_body(nc, tc, n_cores, with_collective,
                           xtr, woc, woc_h, out, wqkv_sb, cst_sb, snt_sb,
                           m01_sb, ident_sb, ones_sb, cc_in, cc_out,
                           load_woc=(rep == 0), denom=denom, look=look,
                           rope4=rope4)

    nc.compile()
    return nc


def _proj_epilogue(nc, work, psC, q_cur, kt_sb, v_sb, cst_sb, snt_sb, ident_sb,
                   scol, T, n, pp, rope4=True):
    if n < 5:
        dst = q_cur[n][:, :] if n < QH else kt_sb[:, scol]
        if rope4:
            # "act6" RoPE: Act casts the PSUM projection to bf16 SBUF once
            # (Act reads PSUM at full rate; DVE does not), then 6 all-bf16
            # SBUF DVE ops run in the fast 2x tier.  cst/snt are
            # partition-doubled [c;c]/[s;s] so every SB+SB operand pair
            # shares its base partition (HW requirement).
            qc = work.tile([128, 512], BF16, tag="qc", bufs=1)
            nc.scalar.copy(qc[:], pp[:])
            t1 = work.tile([64, 512], BF16, tag="rt1", bufs=1)
            t2 = work.tile([64, 512], BF16, tag="rt2", bufs=1)
            t3 = work.tile([64, 512], BF16, tag="rt3", bufs=1)
            t4 = work.tile([64, 512], BF16, tag="rt4", bufs=1)
            nc.vector.tensor_mul(t1[:], qc[0:64, :], cst_sb[0:64, scol])
            nc.vector.tensor_mul(t2[:], qc[64:128, :], snt_sb[64:128, scol])
            nc.vector.tensor_mul(t3[:], qc[0:64, :], snt_sb[0:64, scol])
            nc.vector.tensor_mul(t4[:], qc[64:128, :], cst_sb[64:128, scol])
            nc.vector.tensor_sub(dst[0:64, :], t1[:], t2[:])
            nc.vector.tensor_add(dst[64:128, :], t3[:], t4[:])
        else:
            c = cst_sb[0:64, scol]
            s_ = snt_sb[0:64, scol]
            t1 = work.tile([64, 512], F32, tag="rt1f", bufs=1)
            t2 = work.tile([64, 512], F32, tag="rt2f", bufs=1)
            t3 = work.tile([64, 512], F32, tag="rt3f", bufs=1)
            t4 = work.tile([64, 512], F32, tag="rt4f", bufs=1)
            nc.vector.tensor_mul(t1[:], pp[0:64, :], c)
            nc.vector.tensor_mul(t2[:], pp[64:128, :], s_)
            nc.vector.tensor_mul(t3[:], pp[0:64, :], s_)
            nc.vector.tensor_mul(t4[:], pp[64:128, :], c)
            nc.vector.tensor_sub(dst[0:64, :], t1[:], t2[:])
            nc.vector.tensor_add(dst[64:128, :], t3[:], t4[:])
    else:
        # V: cast to bf16, then PE-transpose 128x128 blocks into [sk, hd]
        vt = work.tile([128, 512], BF16, tag="vt", bufs=1)
        nc.scalar.copy(vt[:], pp[:])
        for j in range(4):
            pst = psC.tile([128, 128], BF16, tag="sc")
            nc.tensor.transpose(pst[:], vt[:, 128 * j: 128 * j + 128],
                                ident_sb[:])
            nc.vector.tensor_copy(
                v_sb[:, 128 * (4 * T + j): 128 * (4 * T + j) + 128],
                pst[:])


def _emit_body(nc, tc, n_cores, with_collective, xtr, woc, woc_h, out, wqkv_sb,
               cst_sb, snt_sb, m01_sb, ident_sb, ones_sb, cc_in, cc_out,
               load_woc=True, denom="pe", look=2, rope4=True):
    # PSUM budget (8 banks): psA 3 + psB 2 + sc ring + sums bank
    #   denom == "pe":  sc bufs=2 + [1,512] sums bank = 8
    #   denom == "dve": sc bufs=3, no sums bank       = 8
    n_sc = 2 if denom == "pe" else 3
    with (
        tc.tile_pool(name="persist", bufs=1) as pers,
        tc.tile_pool(name="qpool", bufs=1) as qpool,
        tc.tile_pool(name="xs", bufs=2) as xpool,
        tc.tile_pool(name="work", bufs=2) as work,
        tc.tile_pool(name="exps", bufs=4) as epool,
        tc.tile_pool(name="outs", bufs=4) as opool,
        tc.tile_pool(name="psA", bufs=3, space="PSUM") as psA,
        tc.tile_pool(name="psB", bufs=2, space="PSUM") as psB,
        tc.tile_pool(name="psC", bufs=n_sc, space="PSUM") as psC,
    ):
        kt_sb = pers.tile([128, S], BF16)
        v_sb = pers.tile([128, S], BF16)   # [sk within blk, hd] per 128-col blk

        WQ_CHUNKS = ((0, 2), (2, 8), (8, 16), (16, 24), (24, 32))

        def wq_tile(d, n):
            for ci, (lo, hi) in enumerate(WQ_CHUNKS):
                if lo <= d < hi:
                    base = 768 * (d - lo) + 128 * n
                    return wqkv_sb[ci][:, base: base + 128]
            raise AssertionError

        def woc_tile(h, j):
            k = h * 8 + j
            return woc_h[k // 16][:, 512 * (k % 16): 512 * (k % 16) + 512]

        def fetch_xt(T, chunks):
            ch = []
            for (lo, hi) in chunks:
                t = xpool.tile([128, 512 * (hi - lo)], BF16, tag="xt", bufs=7,
                               name="xt")
                nc.scalar.dma_start(t[:], xtr[T][:, 512 * lo: 512 * hi])
                ch.append((lo, hi, t))
            return ch

        def emit_wo(t_prev, ogs_prev, splits):
            # wo partial for s-tile t_prev (og's are long ready — zero
            # PE stall):  cc_in[t][sq, d] = sum_h og_h[:, sq]^T @ wo_h[:, d]
            # splits: list of (i_lo, i_hi, out_row_lo) — each range gets its
            # own ReduceScatter so the tail chunk can be small
            for (i_lo, i_hi, out_lo) in splits:
                for i in range(i_lo, i_hi):
                    for j in range(8):
                        pf = psA.tile([128, 512], F32, tag="proj", name="pf")
                        for h in range(QH):
                            nc.tensor.matmul(
                                pf[:],
                                lhsT=ogs_prev[h][:, 128 * i: 128 * i + 128],
                                rhs=woc_tile(h, j),
                                start=(h == 0), stop=(h == QH - 1),
                            )
                        ob = opool.tile([128, 512], BF16, tag="ob", bufs=4,
                                        name="ob")
                        # PSUM->SBUF casts: Act reads PSUM at full rate;
                        # DVE takes the other half (it is idle in this phase;
                        # gpsimd cannot access PSUM at all)
                        if j % 2 == 0:
                            nc.scalar.copy(ob[:], pf[:])
                        else:
                            nc.vector.tensor_copy(ob[:], pf[:])
                        nc.sync.dma_start(
                            cc_in[t_prev][128 * i: 128 * i + 128,
                                          512 * j: 512 * j + 512],
                            ob[:])
                nrow = (i_hi - i_lo) * 128 // n_cores
                co = cc_out[t_prev][32 * (i_lo // 2): 32 * (i_lo // 2) + nrow, :]
                # Collectives must issue from gpsimd (NRT straight-line
                # ordering).  The RS + its out-copy occupy that queue until
                # the collective drains, so the CALLER decides when to emit
                # them (one tile late) to keep partition_broadcasts flowing.

                def issue_rs(t_prev=t_prev, i_lo=i_lo, i_hi=i_hi,
                             out_lo=out_lo, nrow=nrow, co=co):
                    if with_collective:
                        nc.gpsimd.collective_compute(
                            "ReduceScatter",
                            mybir.AluOpType.add,
                            replica_groups=[list(range(n_cores))],
                            ins=[cc_in[t_prev][128 * i_lo: 128 * i_hi, :]],
                            outs=[co],
                        )
                    else:
                        nc.gpsimd.dma_start(co,
                                            cc_in[t_prev][128 * i_lo:
                                                          128 * i_lo + nrow, :])
                    # out-copy on gpsimd too: it waits on the RS, and the
                    # sync queue (next tile's cc_in writes) must not stall
                    nc.gpsimd.dma_start(out[out_lo: out_lo + nrow, :], co)
                pending_rs.append(issue_rs)

        prev = None    # (T-1, ogs of T-1)
        pending_rs = []
        xt_ch = fetch_xt(0, ((0, 2), (2, 8), (8, 16), (16, 24), (24, 32)))

        for T in range(ST):
            def xt_tile(d):
                for lo, hi, t in xt_ch:
                    if lo <= d < hi:
                        return t[:, 512 * (d - lo): 512 * (d - lo) + 512]
                raise AssertionError

            scol = slice(512 * T, 512 * T + 512)
            q_cur = [qpool.tile([128, 512], BF16, tag="q", bufs=8,
                                name=f"q{h}") for h in range(QH)]
            # ---- projections: k, v first, then 4 q heads (d-major sweep,
            # 2 psum banks per pair) ----
            for npair in ((4, 5), (0, 1), (2, 3)):
                pps = {}
                for n in npair:
                    pps[n] = psA.tile([128, 512], F32, tag="proj",
                                      name=f"pp{n}")
                for d in range(NB):
                    for n in npair:
                        nc.tensor.matmul(
                            pps[n][:],
                            lhsT=wq_tile(d, n),
                            rhs=xt_tile(d),
                            start=(d == 0), stop=(d == NB - 1),
                        )
                for n in npair:
                    _proj_epilogue(nc, work, psC, q_cur, kt_sb, v_sb, cst_sb,
                                   snt_sb, ident_sb, scol, T, n, pps[n],
                                   rope4=rope4)

            # prefetch next tile's x^T chunks now: the dma_starts sit on the
            # Act queue BEFORE this tile's exps, so the transfers run during
            # attention instead of after it
            if T + 1 < ST:
                next_xt = fetch_xt(T + 1, ((0, 8), (8, 16), (16, 24), (24, 32)))
            if T == 0 and load_woc:
                # deferred wo-shard load on the SAME queue as the x prefetch:
                # the DMA engine drains transfers in arrival order, so these
                # 4.2 MB land after xtr[0]/xtr[1] instead of contending with
                # the startup-critical projection feeds
                for ch in range(2):
                    nc.scalar.dma_start(
                        woc_h[ch][:],
                        woc[:, 16 * 512 * ch: 16 * 512 * (ch + 1)])

            # ---- attention for this s-tile, 4 heads, 2-deep QK->exp->PV
            # software pipeline; per-block ones-matmul accumulates the
            # softmax denominator in a [1,512] PSUM bank ----
            nsk = 4 * (T + 1)
            ogs = []

            for h in range(QH):
                op = psB.tile([128, 512], F32, tag="outp")
                if denom == "pe":
                    sm = psC.tile([1, 512], F32, tag="sums", bufs=1)

                def qk_block(b):
                    # diagonal blocks r>=1 have their first 128*r columns
                    # fully masked — skip computing them
                    r = b - 4 * T
                    off = 128 * r if r >= 1 else 0
                    N = 512 - off
                    sc = psC.tile([128, 512], F32, tag="sc", bufs=n_sc,
                                  name="sc")
                    nc.tensor.matmul(
                        sc[:, 0:N],
                        lhsT=kt_sb[:, 128 * b: 128 * b + 128],
                        rhs=q_cur[h][:, off: off + N],
                        start=True, stop=True,
                    )
                    e = epool.tile([128, 512], BF16, tag="e", bufs=4 + look,
                                   name="e")
                    nc.scalar.activation(e[:, 0:N], sc[:, 0:N],
                                         mybir.ActivationFunctionType.Exp,
                                         scale=SCALE)
                    if r >= 0:
                        nc.vector.tensor_mul(
                            e[:, 0:N], e[:, 0:N],
                            m01_sb[:, 512 * r + off: 512 * r + 512])
                    return e, off, N

                blocks = {}
                for b in range(min(look, nsk)):
                    blocks[b] = qk_block(b)
                if denom == "dve":
                    ea = epool.tile([128, 512], BF16, tag="ea", bufs=2,
                                    name="ea")
                for b in range(nsk):
                    if b not in blocks:
                        blocks[b] = qk_block(b)
                    e, off, N = blocks.pop(b)
                    if denom == "dve":
                        if b == 0:
                            nc.vector.tensor_copy(ea[:], e[:])
                        else:
                            nc.vector.tensor_add(ea[:, off:512],
                                                 ea[:, off:512], e[:, 0:N])
                    nc.tensor.matmul(op[:, off:512],
                                     lhsT=v_sb[:, 128 * b: 128 * b + 128],
                                     rhs=e[:, 0:N],
                                     start=(b == 0), stop=(b == nsk - 1))
                    if denom == "pe":
                        nc.tensor.matmul(sm[0:1, off:512],
                                         lhsT=ones_sb[:, 0:1],
                                         rhs=e[:, 0:N],
                                         start=(b == 0), stop=(b == nsk - 1))
                    if b + look < nsk:
                        blocks[b + look] = qk_block(b + look)

                # normalization chain: DVE + gpsimd only, PE rolls on
                rz = work.tile([1, 512], F32, tag="rz")
                if denom == "pe":
                    nc.vector.reciprocal(rz[:], sm[0:1, :])
                else:
                    zs = work.tile([128, 512], F32, tag="zs", bufs=2)
                    nc.gpsimd.partition_all_reduce(
                        zs[:], ea[:], channels=128,
                        reduce_op=bass_isa.ReduceOp.add)
                    nc.vector.reciprocal(rz[:], zs[0:1, :])
                rb = work.tile([1, 512], BF16, tag="rb")
                nc.vector.tensor_copy(rb[:], rz[:])
                bcs = work.tile([128, 512], BF16, tag="bcs", bufs=2)
                nc.gpsimd.partition_broadcast(bcs[:], rb[:])
                og = work.tile([128, 512], BF16, tag="og", bufs=8, name="og")
                nc.vector.tensor_mul(og[:], op[:], bcs[:])
                ogs.append(og)

            # ---- wo partial + RS for the PREVIOUS s-tile ----
            if prev is not None:
                t_prev, ogs_prev = prev
                emit_wo(t_prev, ogs_prev, [(0, 4, 64 * t_prev)])
                while pending_rs:
                    pending_rs.pop(0)()
            prev = (T, ogs)
            if T + 1 < ST:
                xt_ch = next_xt

        t_prev, ogs_prev = prev
        emit_wo(t_prev, ogs_prev, [(0, 4, 64 * t_prev)])
        while pending_rs:
            pending_rs.pop(0)()


# ---------------------------------------------------------------------------
# host-side preparation


def _prep_in_maps(x, wq, wk, wv, wo, freqs_cos, freqs_sin, mask):
    x2 = np.asarray(x, np.float32).reshape(S, D)
    perm = np.concatenate([np.arange(0, HD, 2), np.arange(1, HD, 2)])

    xT = np.ascontiguousarray(x2.T).astype(NPBF)                 # [D, S]
    # xtr[T][p, 512*b+f] = xT[128*b+p, 512*T+f]
    xtr = (xT.reshape(NB, 128, ST, 512).transpose(2, 1, 0, 3)
           .reshape(ST, 128, NB * 512))
    xtr = np.ascontiguousarray(xtr)

    # partition-doubled RoPE tables: [c;c] and [s;s]  (4-op RoPE)
    cst1 = np.asarray(freqs_cos, np.float32).T                   # [64, S]
    snt1 = np.asarray(freqs_sin, np.float32).T
    cstn = np.ascontiguousarray(np.concatenate([cst1, cst1], axis=0)).astype(NPBF)
    sntn = np.ascontiguousarray(np.concatenate([snt1, snt1], axis=0)).astype(NPBF)

    mnp = np.asarray(mask, np.float32)
    m01 = np.zeros((128, 4 * 512), np.float32)
    for r in range(4):
        m01[:, 512 * r: 512 * r + 512] = (mnp[0:512, 128 * r: 128 * r + 128].T == 0.0)
    m01 = m01.astype(NPBF)

    identity = np.eye(128, dtype=NPBF)

    wqn = np.asarray(wq, np.float32)
    wkn = np.asarray(wk, np.float32)
    wvn = np.asarray(wv, np.float32)
    won = np.asarray(wo, np.float32)

    in_maps = []
    for c in range(N_CORES):
        wq_c = wqn[:, c * QH * HD:(c + 1) * QH * HD].reshape(D, QH, HD)[:, :, perm]
        wq_c = wq_c.reshape(D, QH * HD)
        wk_c = wkn[:, c * HD:(c + 1) * HD][:, perm]
        wv_c = wvn[:, c * HD:(c + 1) * HD]
        wqkv_c = np.concatenate([wq_c, wk_c, wv_c], axis=1)      # [D, 768]
        wqkv_img = (wqkv_c.reshape(NB, 128, 768).transpose(1, 0, 2)
                    .reshape(128, NB * 768)).astype(NPBF)
        # wo ROW shard: woc[p, h*4096 + d] = wo[c*512 + h*128 + p, d]
        wob = won[c * 512:(c + 1) * 512, :]                      # [512, 4096]
        woc_img = (wob.reshape(QH, 128, D).transpose(1, 0, 2)
                   .reshape(128, QH * D)).astype(NPBF)
        in_maps.append({
            "xtr": xtr,
            "wqkv": np.ascontiguousarray(wqkv_img),
            "woc": np.ascontiguousarray(woc_img),
            "cst": cstn,
            "snt": sntn,
            "m01": np.ascontiguousarray(m01),
            "ident": identity,
        })
    return in_maps


def kernel(x, wq, wk, wv, wo, cache_k, cache_v, freqs_cos, freqs_sin, mask,
           input_indexes):
    """Full-input / full-output entry point.  cache_k/cache_v/input_indexes are
    consumed by the reference semantics (zero cache fully overwritten at
    positions arange(S)), so keys/values equal the fresh projections."""
    in_maps = _prep_in_maps(x, wq, wk, wv, wo, freqs_cos, freqs_sin, mask)
    if "nc" not in _CACHE:
        _CACHE["nc"] = build_kernel()
    nc = _CACHE["nc"]
    res = run_bass_kernel_spmd(nc, in_maps, core_ids=list(range(N_CORES)))
    full = np.empty((S, D), np.float32)
    for c in range(N_CORES):
        o = np.asarray(res.results[c]["out"]).astype(np.float32)  # [256, 4096]
        for T in range(ST):
            full[512 * T + 64 * c: 512 * T + 64 * (c + 1)] = o[64 * T: 64 * (T + 1)]
    return full.reshape(1, S, D)


if __name__ == "__main__":
    rng = np.random.default_rng(0)
    ins = {
        "x": rng.standard_normal((1, S, D), dtype=np.float32),
        "wq": (rng.standard_normal((D, H * HD), dtype=np.float32) * 0.02),
        "wk": (rng.standard_normal((D, KVH * HD), dtype=np.float32) * 0.02),
        "wv": (rng.standard_normal((D, KVH * HD), dtype=np.float32) * 0.02),
        "wo": (rng.standard_normal((H * HD, D), dtype=np.float32) * 0.02),
        "cache_k": np.zeros((1, S, KVH, HD), np.float32),
        "cache_v": np.zeros((1, S, KVH, HD), np.float32),
        "freqs_cos": rng.random((S, HD // 2), dtype=np.float32),
        "freqs_sin": rng.random((S, HD // 2), dtype=np.float32),
        "mask": np.triu(np.full((S, S), -1e9, dtype=np.float32), k=1),
        "input_indexes": np.arange(S, dtype=np.int32),
    }
    o = kernel(**ins)
    print("kernel output", o.shape, o.dtype)


# revision 46
# speedup vs baseline: 1.7095x; 1.7095x over previous
"""Trainium2 8-core tensor-parallel attention kernel (Bass/Tile).

Problem: B=1, S=2048, D=4096, H=32 q-heads, KVH=8 kv-heads, HD=128, causal,
RoPE, GQA.  Sharding: tensor-parallel over heads — each of the 8 cores owns
4 q heads + 1 kv head (wq/wk/wv column shards), computes its heads' attention
output transposed [hd, sq], multiplies by its own wo ROW shard inline per
s-tile to produce a full-width partial [512, 4096], and ReduceScatters each
s-tile partial (add, bf16) across the 8 cores straight into the ExternalOutput.
Host reassembles: full[512*T + 64*c + i] = out_c[64*T + i].

Versus the AllGather formulation this keeps every matmul dependent only on
LOCAL data — the collective is pure background traffic and only the last
s-tile's ReduceScatter (~0.5 MB out) is exposed at the tail.

Compute dtype: bf16 operands / f32 accumulation.
Host-prepared layouts (every DMA a contiguous partition-major image):
  xtr  [4, 128, 32*512]  x^T tiles: xtr[T][p, 512*b+f] = x[512*T+f, 128*b+p]
  wqkv [128, 32*768]     [wq_perm | wk_perm | wv] d-block-major stationary tiles
  woc  [128, 32*512]     wo ROW shard: woc[p, h*4096+d] = wo[c*512+h*128+p, d]
  cst/snt [64, 2048]     RoPE tables transposed (bf16)
  m01  [128, 4*512]      0/1 causal masks for the 4 diagonal block offsets
RoPE trick: wq/wk columns are permuted per head (even channels first) so the
rotation acts on partition halves [0:64]/[64:128] of the projected qT/kT; the
PSUM projection is cast to bf16 SBUF on the Act engine first (Act reads PSUM
at full rate, DVE does not), then 6 all-bf16 SBUF DVE ops run in the fast 2x
tier (cos/sin tables partition-doubled so SB+SB operands share their base
partition).  Softmax in the transposed score layout (scoresT[sk, sq]) without
max-subtraction (|scores| <~ 15).  QK->exp->PV software-pipelined 2 blocks
deep so the Act-engine exp latency never stalls PE.  The denominator is a
bf16 DVE accumulator summed across partitions by a gpsimd
partition_all_reduce (a per-block [1,512] ones-matmul measured ~300us slower
on HW than the cost model claims — never use it); normalization via gpsimd
partition-broadcast of the reciprocal row.
Schedule per tile: proj(T) -> attn(T) with the PREVIOUS tile's 32 wo-partial
psum groups interleaved between attention blocks (PE fills the Act-paced exp
gaps, the PE queue never head-of-line blocks on tile T's own normalization
chain) -> that tile's ReduceScatter.  Only the last tile's wo + RS (~60us)
are exposed at the tail.
"""

import math

import numpy as np
import ml_dtypes

import concourse.bass as bass
import concourse.bass_isa as bass_isa
import concourse.bacc as bacc
import concourse.tile as tile
import concourse.mybir as mybir
from concourse.bass_utils import run_bass_kernel_spmd

BF16 = mybir.dt.bfloat16
F32 = mybir.dt.float32
NPBF = ml_dtypes.bfloat16

N_CORES = 8
S, D, H, KVH, HD = 2048, 4096, 32, 8, 128
QH = H // N_CORES          # 4 q heads per core
NB = D // 128              # 32 contraction blocks
ST = S // 512              # 4 s-tiles
SCALE = 1.0 / math.sqrt(HD)

_CACHE = {}


def build_kernel(n_cores=N_CORES, with_collective=True, reps=1,
                 skip_final=False, ag_splits=4,
                 denom="dve", look=2, rope4=True):
    nc = bacc.Bacc("TRN2", target_bir_lowering=False, debug=False,
                   num_devices=n_cores)

    xtr = nc.declare_dram_parameter("xtr", [ST, 128, NB * 512], BF16, isOutput=False)
    wqkv = nc.declare_dram_parameter("wqkv", [128, NB * 768], BF16, isOutput=False)
    woc = nc.declare_dram_parameter("woc", [128, NB * 512], BF16, isOutput=False)
    cst = nc.declare_dram_parameter("cst", [128, S], BF16, isOutput=False)
    snt = nc.declare_dram_parameter("snt", [128, S], BF16, isOutput=False)
    m01 = nc.declare_dram_parameter("m01", [128, 4 * 512], BF16, isOutput=False)
    ident = nc.declare_dram_parameter("ident", [128, 128], BF16, isOutput=False)
    # RS output: chunk T -> rows [64*T, 64*(T+1))
    out = nc.declare_dram_parameter("out", [ST * 64, D], BF16, isOutput=True)

    with tile.TileContext(nc) as tc:
        with (
            tc.tile_pool(name="const", bufs=1) as constp,
            tc.tile_pool(name="dram", bufs=1, space="DRAM") as dram,
        ):
            # persistent SBUF images, chunked so the first matmuls start after
            # ~0.3 MB of DMA (Tile tracks deps per tile, not per slice)
            WQ_CHUNKS = ((0, 2), (2, 8), (8, 16), (16, 24), (24, 32))
            wqkv_sb = []
            for ci, (lo, hi) in enumerate(WQ_CHUNKS):
                t = constp.tile([128, 768 * (hi - lo)], BF16, name=f"wq_ch{ci}")
                nc.sync.dma_start(t[:], wqkv[:, 768 * lo: 768 * hi])
                wqkv_sb.append(t)
                if ci == 0:
                    cst_sb = constp.tile([128, S], BF16)
                    nc.gpsimd.dma_start(cst_sb[:], cst[:])
                    snt_sb = constp.tile([128, S], BF16)
                    nc.gpsimd.dma_start(snt_sb[:], snt[:])
                    m01_sb = constp.tile([128, 4 * 512], BF16)
                    nc.gpsimd.dma_start(m01_sb[:], m01[:])
                    ident_sb = constp.tile([128, 128], BF16)
                    nc.gpsimd.dma_start(ident_sb[:], ident[:])
                    ones_sb = constp.tile([128, 8], BF16)
                    nc.gpsimd.memset(ones_sb[:], 1.0)
            # wo row-shard halves: tiles allocated here, but the DMAs are
            # emitted inside the body AFTER the T=1 x-prefetch so the startup
            # window isn't contended (first wo use is ~130us in)
            woc_h = [constp.tile([128, 16 * 512], BF16, name=f"woc_h{ch}")
                     for ch in range(2)]

            for rep in range(reps):
                cc_in = [dram.tile([512, D], BF16, name=f"cc_in{rep}_{t}")
                         for t in range(ST)]
                cc_out = [dram.tile([64, D], BF16, name=f"cc_out{rep}_{t}")
                          for t in range(ST)]
                _emit_body(nc, tc, n_cores, with_collective,
                           xtr, woc, woc_h, out, wqkv_sb, cst_sb, snt_sb,
                           m01_sb, ident_sb, ones_sb, cc_in, cc_out,
                           load_woc=(rep == 0), denom=denom, look=look,
                           rope4=rope4)

    nc.compile()
    return nc


def _proj_epilogue(nc, work, psC, q_cur, kt_sb, v_sb, cst_sb, snt_sb, ident_sb,
                   scol, T, n, pp, rope4=True):
    if n < 5:
        dst = q_cur[n][:, :] if n < QH else kt_sb[:, scol]
        if rope4:
            # "act6" RoPE: Act casts the PSUM projection to bf16 SBUF once
            # (Act reads PSUM at full rate; DVE does not), then 6 all-bf16
            # SBUF DVE ops run in the fast 2x tier.  cst/snt are
            # partition-doubled [c;c]/[s;s] so every SB+SB operand pair
            # shares its base partition (HW requirement).
            qc = work.tile([128, 512], BF16, tag="qc", bufs=1)
            nc.scalar.copy(qc[:], pp[:])
            t1 = work.tile([64, 512], BF16, tag="rt1", bufs=1)
            t2 = work.tile([64, 512], BF16, tag="rt2", bufs=1)
            t3 = work.tile([64, 512], BF16, tag="rt3", bufs=1)
            t4 = work.tile([64, 512], BF16, tag="rt4", bufs=1)
            nc.vector.tensor_mul(t1[:], qc[0:64, :], cst_sb[0:64, scol])
            nc.vector.tensor_mul(t2[:], qc[64:128, :], snt_sb[64:128, scol])
            nc.vector.tensor_mul(t3[:], qc[0:64, :], snt_sb[0:64, scol])
            nc.vector.tensor_mul(t4[:], qc[64:128, :], cst_sb[64:128, scol])
            nc.vector.tensor_sub(dst[0:64, :], t1[:], t2[:])
            nc.vector.tensor_add(dst[64:128, :], t3[:], t4[:])
        else:
            c = cst_sb[0:64, scol]
            s_ = snt_sb[0:64, scol]
            t1 = work.tile([64, 512], F32, tag="rt1f", bufs=1)
            t2 = work.tile([64, 512], F32, tag="rt2f", bufs=1)
            t3 = work.tile([64, 512], F32, tag="rt3f", bufs=1)
            t4 = work.tile([64, 512], F32, tag="rt4f", bufs=1)
            nc.vector.tensor_mul(t1[:], pp[0:64, :], c)
            nc.vector.tensor_mul(t2[:], pp[64:128, :], s_)
            nc.vector.tensor_mul(t3[:], pp[0:64, :], s_)
            nc.vector.tensor_mul(t4[:], pp[64:128, :], c)
            nc.vector.tensor_sub(dst[0:64, :], t1[:], t2[:])
            nc.vector.tensor_add(dst[64:128, :], t3[:], t4[:])
    else:
        # V: cast to bf16, then PE-transpose 128x128 blocks into [sk, hd]
        vt = work.tile([128, 512], BF16, tag="vt", bufs=1)
        nc.scalar.copy(vt[:], pp[:])
        for j in range(4):
            pst = psC.tile([128, 128], BF16, tag="sc")
            nc.tensor.transpose(pst[:], vt[:, 128 * j: 128 * j + 128],
                                ident_sb[:])
            nc.vector.tensor_copy(
                v_sb[:, 128 * (4 * T + j): 128 * (4 * T + j) + 128],
                pst[:])


def _emit_body(nc, tc, n_cores, with_collective, xtr, woc, woc_h, out, wqkv_sb,
               cst_sb, snt_sb, m01_sb, ident_sb, ones_sb, cc_in, cc_out,
               load_woc=True, denom="pe", look=2, rope4=True):
    # PSUM budget (8 banks): psA 3 + psB 2 + sc ring + sums bank
    #   denom == "pe":  sc bufs=2 + [1,512] sums bank = 8
    #   denom == "dve": sc bufs=3, no sums bank       = 8
    n_sc = 2 if denom == "pe" else 3
    with (
        tc.tile_pool(name="persist", bufs=1) as pers,
        tc.tile_pool(name="qpool", bufs=1) as qpool,
        tc.tile_pool(name="xs", bufs=2) as xpool,
        tc.tile_pool(name="work", bufs=2) as work,
        tc.tile_pool(name="exps", bufs=4) as epool,
        tc.tile_pool(name="outs", bufs=4) as opool,
        tc.tile_pool(name="psA", bufs=3, space="PSUM") as psA,
        tc.tile_pool(name="psB", bufs=2, space="PSUM") as psB,
        tc.tile_pool(name="psC", bufs=n_sc, space="PSUM") as psC,
    ):
        kt_sb = pers.tile([128, S], BF16)
        v_sb = pers.tile([128, S], BF16)   # [sk within blk, hd] per 128-col blk

        WQ_CHUNKS = ((0, 2), (2, 8), (8, 16), (16, 24), (24, 32))

        def wq_tile(d, n):
            for ci, (lo, hi) in enumerate(WQ_CHUNKS):
                if lo <= d < hi:
                    base = 768 * (d - lo) + 128 * n
                    return wqkv_sb[ci][:, base: base + 128]
            raise AssertionError

        def woc_tile(h, j):
            k = h * 8 + j
            return woc_h[k // 16][:, 512 * (k % 16): 512 * (k % 16) + 512]

        def fetch_xt(T, chunks):
            ch = []
            for (lo, hi) in chunks:
                t = xpool.tile([128, 512 * (hi - lo)], BF16, tag="xt", bufs=7,
                               name="xt")
                nc.scalar.dma_start(t[:], xtr[T][:, 512 * lo: 512 * hi])
                ch.append((lo, hi, t))
            return ch

        def wo_emitters(t_prev, ogs_prev, splits):
            """Closures emitting wo-partial work for s-tile t_prev:
            32 psum-group closures (4 matmuls + cast + cc_in DMA each) plus a
            ReduceScatter closure after each split's groups.  The caller
            interleaves them between attention blocks of the NEXT tile so PE
            fills the Act-paced exp gaps instead of idling."""
            ems = []
            for (i_lo, i_hi, out_lo) in splits:
                for i in range(i_lo, i_hi):
                    for j in range(8):
                        def group(i=i, j=j):
                            pf = psA.tile([128, 512], F32, tag="proj",
                                          name="pf")
                            for h in range(QH):
                                nc.tensor.matmul(
                                    pf[:],
                                    lhsT=ogs_prev[h][:, 128 * i: 128 * i + 128],
                                    rhs=woc_tile(h, j),
                                    start=(h == 0), stop=(h == QH - 1),
                                )
                            ob = opool.tile([128, 512], BF16, tag="ob",
                                            bufs=4, name="ob")
                            # PSUM->SBUF casts: Act reads PSUM at full rate;
                            # DVE takes the other half (gpsimd cannot access
                            # PSUM at all)
                            if j % 2 == 0:
                                nc.scalar.copy(ob[:], pf[:])
                            else:
                                nc.vector.tensor_copy(ob[:], pf[:])
                            nc.sync.dma_start(
                                cc_in[t_prev][128 * i: 128 * i + 128,
                                              512 * j: 512 * j + 512],
                                ob[:])
                        ems.append(group)
                nrow = (i_hi - i_lo) * 128 // n_cores
                co = cc_out[t_prev][32 * (i_lo // 2): 32 * (i_lo // 2) + nrow, :]

                def issue_rs(t_prev=t_prev, i_lo=i_lo, i_hi=i_hi,
                             out_lo=out_lo, nrow=nrow, co=co):
                    # collectives must issue from gpsimd (NRT straight-line
                    # ordering); the out-copy also goes there so the sync
                    # queue (cc_in writes) never waits on a collective
                    if with_collective:
                        nc.gpsimd.collective_compute(
                            "ReduceScatter",
                            mybir.AluOpType.add,
                            replica_groups=[list(range(n_cores))],
                            ins=[cc_in[t_prev][128 * i_lo: 128 * i_hi, :]],
                            outs=[co],
                        )
                    else:
                        nc.gpsimd.dma_start(co,
                                            cc_in[t_prev][128 * i_lo:
                                                          128 * i_lo + nrow, :])
                    nc.gpsimd.dma_start(out[out_lo: out_lo + nrow, :], co)
                ems.append(issue_rs)
            return ems

        wo_work = []   # staged wo emitters of the previous s-tile
        xt_ch = fetch_xt(0, ((0, 2), (2, 8), (8, 16), (16, 24), (24, 32)))

        for T in range(ST):
            def xt_tile(d):
                for lo, hi, t in xt_ch:
                    if lo <= d < hi:
                        return t[:, 512 * (d - lo): 512 * (d - lo) + 512]
                raise AssertionError

            scol = slice(512 * T, 512 * T + 512)
            q_cur = [qpool.tile([128, 512], BF16, tag="q", bufs=8,
                                name=f"q{h}") for h in range(QH)]
            # ---- projections: k, v first, then 4 q heads (d-major sweep,
            # 2 psum banks per pair) ----
            for npair in ((4, 5), (0, 1), (2, 3)):
                pps = {}
                for n in npair:
                    pps[n] = psA.tile([128, 512], F32, tag="proj",
                                      name=f"pp{n}")
                for d in range(NB):
                    for n in npair:
                        nc.tensor.matmul(
                            pps[n][:],
                            lhsT=wq_tile(d, n),
                            rhs=xt_tile(d),
                            start=(d == 0), stop=(d == NB - 1),
                        )
                for n in npair:
                    _proj_epilogue(nc, work, psC, q_cur, kt_sb, v_sb, cst_sb,
                                   snt_sb, ident_sb, scol, T, n, pps[n],
                                   rope4=rope4)

            # prefetch next tile's x^T chunks now: the dma_starts sit on the
            # Act queue BEFORE this tile's exps, so the transfers run during
            # attention instead of after it
            if T + 1 < ST:
                next_xt = fetch_xt(T + 1, ((0, 8), (8, 16), (16, 24), (24, 32)))
            if T == 0 and load_woc:
                # deferred wo-shard load on the SAME queue as the x prefetch:
                # the DMA engine drains transfers in arrival order, so these
                # 4.2 MB land after xtr[0]/xtr[1] instead of contending with
                # the startup-critical projection feeds
                for ch in range(2):
                    nc.scalar.dma_start(
                        woc_h[ch][:],
                        woc[:, 16 * 512 * ch: 16 * 512 * (ch + 1)])

            # ---- attention for this s-tile, 4 heads, 2-deep QK->exp->PV
            # software pipeline, with the PREVIOUS tile's wo-partial groups
            # interleaved between blocks (PE fills Act-paced exp gaps) ----
            nsk = 4 * (T + 1)
            ogs = []
            wo_q = list(wo_work)
            wo_work = []
            blocks_total = nsk * QH
            blocks_done = 0
            emitted = 0

            for h in range(QH):
                op = psB.tile([128, 512], F32, tag="outp")
                if denom == "pe":
                    sm = psC.tile([1, 512], F32, tag="sums", bufs=1)

                def qk_block(b):
                    # diagonal blocks r>=1 have their first 128*r columns
                    # fully masked — skip computing them
                    r = b - 4 * T
                    off = 128 * r if r >= 1 else 0
                    N = 512 - off
                    sc = psC.tile([128, 512], F32, tag="sc", bufs=n_sc,
                                  name="sc")
                    nc.tensor.matmul(
                        sc[:, 0:N],
                        lhsT=kt_sb[:, 128 * b: 128 * b + 128],
                        rhs=q_cur[h][:, off: off + N],
                        start=True, stop=True,
                    )
                    e = epool.tile([128, 512], BF16, tag="e", bufs=4 + look,
                                   name="e")
                    nc.scalar.activation(e[:, 0:N], sc[:, 0:N],
                                         mybir.ActivationFunctionType.Exp,
                                         scale=SCALE)
                    if r >= 0:
                        nc.vector.tensor_mul(
                            e[:, 0:N], e[:, 0:N],
                            m01_sb[:, 512 * r + off: 512 * r + 512])
                    return e, off, N

                blocks = {}
                for b in range(min(look, nsk)):
                    blocks[b] = qk_block(b)
                if denom == "dve":
                    ea = epool.tile([128, 512], BF16, tag="ea", bufs=2,
                                    name="ea")
                for b in range(nsk):
                    if b not in blocks:
                        blocks[b] = qk_block(b)
                    e, off, N = blocks.pop(b)
                    if denom == "dve":
                        if b == 0:
                            nc.vector.tensor_copy(ea[:], e[:])
                        else:
                            nc.vector.tensor_add(ea[:, off:512],
                                                 ea[:, off:512], e[:, 0:N])
                    nc.tensor.matmul(op[:, off:512],
                                     lhsT=v_sb[:, 128 * b: 128 * b + 128],
                                     rhs=e[:, 0:N],
                                     start=(b == 0), stop=(b == nsk - 1))
                    if denom == "pe":
                        nc.tensor.matmul(sm[0:1, off:512],
                                         lhsT=ones_sb[:, 0:1],
                                         rhs=e[:, 0:N],
                                         start=(b == 0), stop=(b == nsk - 1))
                    if b + look < nsk:
                        blocks[b + look] = qk_block(b + look)
                    blocks_done += 1
                    target = len(wo_q) * blocks_done // blocks_total
                    while emitted < target:
                        wo_q[emitted]()
                        emitted += 1

                # normalization chain: DVE + gpsimd only, PE rolls on
                rz = work.tile([1, 512], F32, tag="rz")
                if denom == "pe":
                    nc.vector.reciprocal(rz[:], sm[0:1, :])
                else:
                    zs = work.tile([128, 512], F32, tag="zs", bufs=2)
                    nc.gpsimd.partition_all_reduce(
                        zs[:], ea[:], channels=128,
                        reduce_op=bass_isa.ReduceOp.add)
                    nc.vector.reciprocal(rz[:], zs[0:1, :])
                rb = work.tile([1, 512], BF16, tag="rb")
                nc.vector.tensor_copy(rb[:], rz[:])
                bcs = work.tile([128, 512], BF16, tag="bcs", bufs=2)
                nc.gpsimd.partition_broadcast(bcs[:], rb[:])
                og = work.tile([128, 512], BF16, tag="og", bufs=8, name="og")
                nc.vector.tensor_mul(og[:], op[:], bcs[:])
                ogs.append(og)

            # drain any leftover wo work of the previous tile, then stage
            # this tile's wo for interleaving into the next tile's attention
            for em in wo_q[emitted:]:
                em()
            wo_work = wo_emitters(T, ogs, [(0, 4, 64 * T)])
            if T + 1 < ST:
                xt_ch = next_xt

        # last tile has no following attention to interleave with
        for em in wo_work:
            em()


# ---------------------------------------------------------------------------
# host-side preparation


def _prep_in_maps(x, wq, wk, wv, wo, freqs_cos, freqs_sin, mask):
    x2 = np.asarray(x, np.float32).reshape(S, D)
    perm = np.concatenate([np.arange(0, HD, 2), np.arange(1, HD, 2)])

    xT = np.ascontiguousarray(x2.T).astype(NPBF)                 # [D, S]
    # xtr[T][p, 512*b+f] = xT[128*b+p, 512*T+f]
    xtr = (xT.reshape(NB, 128, ST, 512).transpose(2, 1, 0, 3)
           .reshape(ST, 128, NB * 512))
    xtr = np.ascontiguousarray(xtr)

    # partition-doubled RoPE tables: [c;c] and [s;s]  (4-op RoPE)
    cst1 = np.asarray(freqs_cos, np.float32).T                   # [64, S]
    snt1 = np.asarray(freqs_sin, np.float32).T
    cstn = np.ascontiguousarray(np.concatenate([cst1, cst1], axis=0)).astype(NPBF)
    sntn = np.ascontiguousarray(np.concatenate([snt1, snt1], axis=0)).astype(NPBF)

    mnp = np.asarray(mask, np.float32)
    m01 = np.zeros((128, 4 * 512), np.float32)
    for r in range(4):
        m01[:, 512 * r: 512 * r + 512] = (mnp[0:512, 128 * r: 128 * r + 128].T == 0.0)
    m01 = m01.astype(NPBF)

    identity = np.eye(128, dtype=NPBF)

    wqn = np.asarray(wq, np.float32)
    wkn = np.asarray(wk, np.float32)
    wvn = np.asarray(wv, np.float32)
    won = np.asarray(wo, np.float32)

    in_maps = []
    for c in range(N_CORES):
        wq_c = wqn[:, c * QH * HD:(c + 1) * QH * HD].reshape(D, QH, HD)[:, :, perm]
        wq_c = wq_c.reshape(D, QH * HD)
        wk_c = wkn[:, c * HD:(c + 1) * HD][:, perm]
        wv_c = wvn[:, c * HD:(c + 1) * HD]
        wqkv_c = np.concatenate([wq_c, wk_c, wv_c], axis=1)      # [D, 768]
        wqkv_img = (wqkv_c.reshape(NB, 128, 768).transpose(1, 0, 2)
                    .reshape(128, NB * 768)).astype(NPBF)
        # wo ROW shard: woc[p, h*4096 + d] = wo[c*512 + h*128 + p, d]
        wob = won[c * 512:(c + 1) * 512, :]                      # [512, 4096]
        woc_img = (wob.reshape(QH, 128, D).transpose(1, 0, 2)
                   .reshape(128, QH * D)).astype(NPBF)
        in_maps.append({
            "xtr": xtr,
            "wqkv": np.ascontiguousarray(wqkv_img),
            "woc": np.ascontiguousarray(woc_img),
            "cst": cstn,
            "snt": sntn,
            "m01": np.ascontiguousarray(m01),
            "ident": identity,
        })
    return in_maps


def kernel(x, wq, wk, wv, wo, cache_k, cache_v, freqs_cos, freqs_sin, mask,
           input_indexes):
    """Full-input / full-output entry point.  cache_k/cache_v/input_indexes are
    consumed by the reference semantics (zero cache fully overwritten at
    positions arange(S)), so keys/values equal the fresh projections."""
    in_maps = _prep_in_maps(x, wq, wk, wv, wo, freqs_cos, freqs_sin, mask)
    if "nc" not in _CACHE:
        _CACHE["nc"] = build_kernel()
    nc = _CACHE["nc"]
    res = run_bass_kernel_spmd(nc, in_maps, core_ids=list(range(N_CORES)))
    full = np.empty((S, D), np.float32)
    for c in range(N_CORES):
        o = np.asarray(res.results[c]["out"]).astype(np.float32)  # [256, 4096]
        for T in range(ST):
            full[512 * T + 64 * c: 512 * T + 64 * (c + 1)] = o[64 * T: 64 * (T + 1)]
    return full.reshape(1, S, D)


if __name__ == "__main__":
    rng = np.random.default_rng(0)
    ins = {
        "x": rng.standard_normal((1, S, D), dtype=np.float32),
        "wq": (rng.standard_normal((D, H * HD), dtype=np.float32) * 0.02),
        "wk": (rng.standard_normal((D, KVH * HD), dtype=np.float32) * 0.02),
        "wv": (rng.standard_normal((D, KVH * HD), dtype=np.float32) * 0.02),
        "wo": (rng.standard_normal((H * HD, D), dtype=np.float32) * 0.02),
        "cache_k": np.zeros((1, S, KVH, HD), np.float32),
        "cache_v": np.zeros((1, S, KVH, HD), np.float32),
        "freqs_cos": rng.random((S, HD // 2), dtype=np.float32),
        "freqs_sin": rng.random((S, HD // 2), dtype=np.float32),
        "mask": np.triu(np.full((S, S), -1e9, dtype=np.float32), k=1),
        "input_indexes": np.arange(S, dtype=np.int32),
    }
    o = kernel(**ins)
    print("kernel output", o.shape, o.dtype)


# revision 49
# speedup vs baseline: 1.8325x; 1.0719x over previous
"""Trainium2 8-core tensor-parallel attention kernel (Bass/Tile).

Problem: B=1, S=2048, D=4096, H=32 q-heads, KVH=8 kv-heads, HD=128, causal,
RoPE, GQA.  Sharding: tensor-parallel over heads — each of the 8 cores owns
4 q heads + 1 kv head (wq/wk/wv column shards), computes its heads' attention
output transposed [hd, sq], multiplies by its own wo ROW shard inline per
s-tile to produce a full-width partial [512, 4096], and ReduceScatters each
s-tile partial (add, bf16) across the 8 cores straight into the ExternalOutput.
Host reassembles: full[512*T + 64*c + i] = out_c[64*T + i].

Versus the AllGather formulation this keeps every matmul dependent only on
LOCAL data — the collective is pure background traffic and only the last
s-tile's ReduceScatter (~0.5 MB out) is exposed at the tail.

Compute dtype: bf16 operands / f32 accumulation.
Host-prepared layouts (every DMA a contiguous partition-major image):
  xtr  [4, 128, 32*512]  x^T tiles: xtr[T][p, 512*b+f] = x[512*T+f, 128*b+p]
  wqkv [128, 32*768]     [wq_perm | wk_perm | wv] d-block-major stationary tiles
  woc  [128, 32*512]     wo ROW shard: woc[p, h*4096+d] = wo[c*512+h*128+p, d]
  cst/snt [64, 2048]     RoPE tables transposed (bf16)
  m01  [128, 4*512]      0/1 causal masks for the 4 diagonal block offsets
RoPE trick: wq/wk columns are permuted per head (even channels first) so the
rotation acts on partition halves [0:64]/[64:128] of the projected qT/kT; the
PSUM projection is cast to bf16 SBUF on the Act engine first (Act reads PSUM
at full rate, DVE does not), then 6 all-bf16 SBUF DVE ops run in the fast 2x
tier (cos/sin tables partition-doubled so SB+SB operands share their base
partition).  Softmax in the transposed score layout (scoresT[sk, sq]) without
max-subtraction (|scores| <~ 15).  QK->exp->PV software-pipelined 2 blocks
deep so the Act-engine exp latency never stalls PE.  The denominator is a
bf16 DVE accumulator summed across partitions by a gpsimd
partition_all_reduce (a per-block [1,512] ones-matmul measured ~300us slower
on HW than the cost model claims — never use it); normalization via gpsimd
partition-broadcast of the reciprocal row.
Schedule per tile: proj(T) -> attn(T) with the PREVIOUS tile's 32 wo-partial
psum groups interleaved between attention blocks (PE fills the Act-paced exp
gaps, the PE queue never head-of-line blocks on tile T's own normalization
chain) -> that tile's ReduceScatter.  Only the last tile's wo + RS (~60us)
are exposed at the tail.
"""

import math

import numpy as np
import ml_dtypes

import concourse.bass as bass
import concourse.bass_isa as bass_isa
import concourse.bacc as bacc
import concourse.tile as tile
import concourse.mybir as mybir
from concourse.bass_utils import run_bass_kernel_spmd

BF16 = mybir.dt.bfloat16
F32 = mybir.dt.float32
NPBF = ml_dtypes.bfloat16

N_CORES = 8
S, D, H, KVH, HD = 2048, 4096, 32, 8, 128
QH = H // N_CORES          # 4 q heads per core
NB = D // 128              # 32 contraction blocks
ST = S // 512              # 4 s-tiles
SCALE = 1.0 / math.sqrt(HD)

_CACHE = {}


def build_kernel(n_cores=N_CORES, with_collective=True, reps=1,
                 skip_final=False, ag_splits=4,
                 denom="dve", look=3, rope4=True):
    nc = bacc.Bacc("TRN2", target_bir_lowering=False, debug=False,
                   num_devices=n_cores)

    xtr = nc.declare_dram_parameter("xtr", [ST, 128, NB * 512], BF16, isOutput=False)
    wqkv = nc.declare_dram_parameter("wqkv", [128, NB * 768], BF16, isOutput=False)
    woc = nc.declare_dram_parameter("woc", [128, NB * 512], BF16, isOutput=False)
    cst = nc.declare_dram_parameter("cst", [128, S], BF16, isOutput=False)
    snt = nc.declare_dram_parameter("snt", [128, S], BF16, isOutput=False)
    m01 = nc.declare_dram_parameter("m01", [128, 4 * 512], BF16, isOutput=False)
    ident = nc.declare_dram_parameter("ident", [128, 128], BF16, isOutput=False)
    # RS output: chunk T -> rows [64*T, 64*(T+1))
    out = nc.declare_dram_parameter("out", [ST * 64, D], BF16, isOutput=True)

    with tile.TileContext(nc) as tc:
        with (
            tc.tile_pool(name="const", bufs=1) as constp,
            tc.tile_pool(name="dram", bufs=1, space="DRAM") as dram,
        ):
            # persistent SBUF images, chunked so the first matmuls start after
            # ~0.3 MB of DMA (Tile tracks deps per tile, not per slice)
            WQ_CHUNKS = ((0, 2), (2, 8), (8, 16), (16, 24), (24, 32))
            wqkv_sb = []
            for ci, (lo, hi) in enumerate(WQ_CHUNKS):
                t = constp.tile([128, 768 * (hi - lo)], BF16, name=f"wq_ch{ci}")
                nc.sync.dma_start(t[:], wqkv[:, 768 * lo: 768 * hi])
                wqkv_sb.append(t)
                if ci == 0:
                    cst_sb = constp.tile([128, S], BF16)
                    nc.gpsimd.dma_start(cst_sb[:], cst[:])
                    snt_sb = constp.tile([128, S], BF16)
                    nc.gpsimd.dma_start(snt_sb[:], snt[:])
                    m01_sb = constp.tile([128, 4 * 512], BF16)
                    nc.gpsimd.dma_start(m01_sb[:], m01[:])
                    ident_sb = constp.tile([128, 128], BF16)
                    nc.gpsimd.dma_start(ident_sb[:], ident[:])
                    ones_sb = constp.tile([128, 8], BF16)
                    nc.gpsimd.memset(ones_sb[:], 1.0)
            # wo row-shard halves: tiles allocated here, but the DMAs are
            # emitted inside the body AFTER the T=1 x-prefetch so the startup
            # window isn't contended (first wo use is ~130us in)
            woc_h = [constp.tile([128, 16 * 512], BF16, name=f"woc_h{ch}")
                     for ch in range(2)]

            for rep in range(reps):
                cc_in = [dram.tile([512, D], BF16, name=f"cc_in{rep}_{t}")
                         for t in range(ST)]
                cc_out = [dram.tile([64, D], BF16, name=f"cc_out{rep}_{t}")
                          for t in range(ST)]
                _emit_body(nc, tc, n_cores, with_collective,
                           xtr, woc, woc_h, out, wqkv_sb, cst_sb, snt_sb,
                           m01_sb, ident_sb, ones_sb, cc_in, cc_out,
                           load_woc=(rep == 0), denom=denom, look=look,
                           rope4=rope4)

    nc.compile()
    return nc


def _proj_epilogue(nc, work, psC, q_cur, kt_sb, v_sb, cst_sb, snt_sb, ident_sb,
                   scol, T, n, pp, rope4=True):
    if n < 5:
        dst = q_cur[n][:, :] if n < QH else kt_sb[:, scol]
        if rope4:
            # "act6" RoPE: Act casts the PSUM projection to bf16 SBUF once
            # (Act reads PSUM at full rate; DVE does not), then 6 all-bf16
            # SBUF DVE ops run in the fast 2x tier.  cst/snt are
            # partition-doubled [c;c]/[s;s] so every SB+SB operand pair
            # shares its base partition (HW requirement).
            qc = work.tile([128, 512], BF16, tag="qc", bufs=1)
            nc.scalar.copy(qc[:], pp[:])
            t1 = work.tile([64, 512], BF16, tag="rt1", bufs=1)
            t2 = work.tile([64, 512], BF16, tag="rt2", bufs=1)
            t3 = work.tile([64, 512], BF16, tag="rt3", bufs=1)
            t4 = work.tile([64, 512], BF16, tag="rt4", bufs=1)
            nc.vector.tensor_mul(t1[:], qc[0:64, :], cst_sb[0:64, scol])
            nc.vector.tensor_mul(t2[:], qc[64:128, :], snt_sb[64:128, scol])
            nc.vector.tensor_mul(t3[:], qc[0:64, :], snt_sb[0:64, scol])
            nc.vector.tensor_mul(t4[:], qc[64:128, :], cst_sb[64:128, scol])
            nc.vector.tensor_sub(dst[0:64, :], t1[:], t2[:])
            nc.vector.tensor_add(dst[64:128, :], t3[:], t4[:])
        else:
            c = cst_sb[0:64, scol]
            s_ = snt_sb[0:64, scol]
            t1 = work.tile([64, 512], F32, tag="rt1f", bufs=1)
            t2 = work.tile([64, 512], F32, tag="rt2f", bufs=1)
            t3 = work.tile([64, 512], F32, tag="rt3f", bufs=1)
            t4 = work.tile([64, 512], F32, tag="rt4f", bufs=1)
            nc.vector.tensor_mul(t1[:], pp[0:64, :], c)
            nc.vector.tensor_mul(t2[:], pp[64:128, :], s_)
            nc.vector.tensor_mul(t3[:], pp[0:64, :], s_)
            nc.vector.tensor_mul(t4[:], pp[64:128, :], c)
            nc.vector.tensor_sub(dst[0:64, :], t1[:], t2[:])
            nc.vector.tensor_add(dst[64:128, :], t3[:], t4[:])
    else:
        # V: cast to bf16, then PE-transpose 128x128 blocks into [sk, hd]
        vt = work.tile([128, 512], BF16, tag="vt", bufs=1)
        nc.scalar.copy(vt[:], pp[:])
        for j in range(4):
            pst = psC.tile([128, 128], BF16, tag="sc")
            nc.tensor.transpose(pst[:], vt[:, 128 * j: 128 * j + 128],
                                ident_sb[:])
            nc.vector.tensor_copy(
                v_sb[:, 128 * (4 * T + j): 128 * (4 * T + j) + 128],
                pst[:])


def _emit_body(nc, tc, n_cores, with_collective, xtr, woc, woc_h, out, wqkv_sb,
               cst_sb, snt_sb, m01_sb, ident_sb, ones_sb, cc_in, cc_out,
               load_woc=True, denom="pe", look=2, rope4=True):
    # PSUM budget (8 banks): psA 3 + psB 2 + sc ring + sums bank
    #   denom == "pe":  sc bufs=2 + [1,512] sums bank = 8
    #   denom == "dve": sc bufs=3, no sums bank       = 8
    n_sc = 2 if denom == "pe" else 3
    with (
        tc.tile_pool(name="persist", bufs=1) as pers,
        tc.tile_pool(name="qpool", bufs=1) as qpool,
        tc.tile_pool(name="xs", bufs=2) as xpool,
        tc.tile_pool(name="work", bufs=2) as work,
        tc.tile_pool(name="exps", bufs=4) as epool,
        tc.tile_pool(name="outs", bufs=4) as opool,
        tc.tile_pool(name="psA", bufs=3, space="PSUM") as psA,
        tc.tile_pool(name="psB", bufs=2, space="PSUM") as psB,
        tc.tile_pool(name="psC", bufs=n_sc, space="PSUM") as psC,
    ):
        kt_sb = pers.tile([128, S], BF16)
        v_sb = pers.tile([128, S], BF16)   # [sk within blk, hd] per 128-col blk

        WQ_CHUNKS = ((0, 2), (2, 8), (8, 16), (16, 24), (24, 32))

        def wq_tile(d, n):
            for ci, (lo, hi) in enumerate(WQ_CHUNKS):
                if lo <= d < hi:
                    base = 768 * (d - lo) + 128 * n
                    return wqkv_sb[ci][:, base: base + 128]
            raise AssertionError

        def woc_tile(h, j):
            k = h * 8 + j
            return woc_h[k // 16][:, 512 * (k % 16): 512 * (k % 16) + 512]

        def fetch_xt(T, chunks):
            ch = []
            for (lo, hi) in chunks:
                t = xpool.tile([128, 512 * (hi - lo)], BF16, tag="xt", bufs=7,
                               name="xt")
                nc.scalar.dma_start(t[:], xtr[T][:, 512 * lo: 512 * hi])
                ch.append((lo, hi, t))
            return ch

        def wo_emitters(t_prev, ogs_prev, splits):
            """Closures emitting wo-partial work for s-tile t_prev:
            32 psum-group closures (4 matmuls + cast + cc_in DMA each) plus a
            ReduceScatter closure after each split's groups.  The caller
            interleaves them between attention blocks of the NEXT tile so PE
            fills the Act-paced exp gaps instead of idling."""
            ems = []
            for (i_lo, i_hi, out_lo) in splits:
                for i in range(i_lo, i_hi):
                    for j in range(8):
                        def group(i=i, j=j):
                            pf = psA.tile([128, 512], F32, tag="proj",
                                          name="pf")
                            for h in range(QH):
                                nc.tensor.matmul(
                                    pf[:],
                                    lhsT=ogs_prev[h][:, 128 * i: 128 * i + 128],
                                    rhs=woc_tile(h, j),
                                    start=(h == 0), stop=(h == QH - 1),
                                )
                            ob = opool.tile([128, 512], BF16, tag="ob",
                                            bufs=4, name="ob")
                            # PSUM->SBUF casts: Act reads PSUM at full rate;
                            # DVE takes the other half (gpsimd cannot access
                            # PSUM at all)
                            if j % 2 == 0:
                                nc.scalar.copy(ob[:], pf[:])
                            else:
                                nc.vector.tensor_copy(ob[:], pf[:])
                            nc.sync.dma_start(
                                cc_in[t_prev][128 * i: 128 * i + 128,
                                              512 * j: 512 * j + 512],
                                ob[:])
                        ems.append(group)
                nrow = (i_hi - i_lo) * 128 // n_cores
                co = cc_out[t_prev][32 * (i_lo // 2): 32 * (i_lo // 2) + nrow, :]

                def issue_rs(t_prev=t_prev, i_lo=i_lo, i_hi=i_hi,
                             out_lo=out_lo, nrow=nrow, co=co):
                    # collectives must issue from gpsimd (NRT straight-line
                    # ordering); the out-copy also goes there so the sync
                    # queue (cc_in writes) never waits on a collective
                    if with_collective:
                        nc.gpsimd.collective_compute(
                            "ReduceScatter",
                            mybir.AluOpType.add,
                            replica_groups=[list(range(n_cores))],
                            ins=[cc_in[t_prev][128 * i_lo: 128 * i_hi, :]],
                            outs=[co],
                        )
                    else:
                        nc.gpsimd.dma_start(co,
                                            cc_in[t_prev][128 * i_lo:
                                                          128 * i_lo + nrow, :])
                    nc.gpsimd.dma_start(out[out_lo: out_lo + nrow, :], co)
                ems.append(issue_rs)
            return ems

        wo_work = []   # staged wo emitters of the previous s-tile
        xt_ch = fetch_xt(0, ((0, 2), (2, 8), (8, 16), (16, 24), (24, 32)))

        for T in range(ST):
            def xt_tile(d):
                for lo, hi, t in xt_ch:
                    if lo <= d < hi:
                        return t[:, 512 * (d - lo): 512 * (d - lo) + 512]
                raise AssertionError

            scol = slice(512 * T, 512 * T + 512)
            q_cur = [qpool.tile([128, 512], BF16, tag="q", bufs=8,
                                name=f"q{h}") for h in range(QH)]
            # ---- projections: k, v first, then 4 q heads (d-major sweep,
            # 2 psum banks per pair) ----
            for npair in ((4, 5), (0, 1), (2, 3)):
                pps = {}
                for n in npair:
                    pps[n] = psA.tile([128, 512], F32, tag="proj",
                                      name=f"pp{n}")
                for d in range(NB):
                    for n in npair:
                        nc.tensor.matmul(
                            pps[n][:],
                            lhsT=wq_tile(d, n),
                            rhs=xt_tile(d),
                            start=(d == 0), stop=(d == NB - 1),
                        )
                for n in npair:
                    _proj_epilogue(nc, work, psC, q_cur, kt_sb, v_sb, cst_sb,
                                   snt_sb, ident_sb, scol, T, n, pps[n],
                                   rope4=rope4)

            # prefetch next tile's x^T chunks now: the dma_starts sit on the
            # Act queue BEFORE this tile's exps, so the transfers run during
            # attention instead of after it
            if T + 1 < ST:
                next_xt = fetch_xt(T + 1, ((0, 8), (8, 16), (16, 24), (24, 32)))
            if T == 0 and load_woc:
                # deferred wo-shard load on the SAME queue as the x prefetch:
                # the DMA engine drains transfers in arrival order, so these
                # 4.2 MB land after xtr[0]/xtr[1] instead of contending with
                # the startup-critical projection feeds
                for ch in range(2):
                    nc.scalar.dma_start(
                        woc_h[ch][:],
                        woc[:, 16 * 512 * ch: 16 * 512 * (ch + 1)])

            # ---- attention for this s-tile, 4 heads, 2-deep QK->exp->PV
            # software pipeline, with the PREVIOUS tile's wo-partial groups
            # interleaved between blocks (PE fills Act-paced exp gaps) ----
            nsk = 4 * (T + 1)
            ogs = []
            wo_q = list(wo_work)
            wo_work = []
            blocks_total = nsk * QH
            blocks_done = 0
            emitted = 0

            for h in range(QH):
                op = psB.tile([128, 512], F32, tag="outp")
                if denom == "pe":
                    sm = psC.tile([1, 512], F32, tag="sums", bufs=1)

                def qk_block(b):
                    # diagonal blocks r>=1 have their first 128*r columns
                    # fully masked — skip computing them
                    r = b - 4 * T
                    off = 128 * r if r >= 1 else 0
                    N = 512 - off
                    sc = psC.tile([128, 512], F32, tag="sc", bufs=n_sc,
                                  name="sc")
                    nc.tensor.matmul(
                        sc[:, 0:N],
                        lhsT=kt_sb[:, 128 * b: 128 * b + 128],
                        rhs=q_cur[h][:, off: off + N],
                        start=True, stop=True,
                    )
                    e = epool.tile([128, 512], BF16, tag="e", bufs=4 + look,
                                   name="e")
                    nc.scalar.activation(e[:, 0:N], sc[:, 0:N],
                                         mybir.ActivationFunctionType.Exp,
                                         scale=SCALE)
                    if r >= 0:
                        nc.vector.tensor_mul(
                            e[:, 0:N], e[:, 0:N],
                            m01_sb[:, 512 * r + off: 512 * r + 512])
                    return e, off, N

                blocks = {}
                for b in range(min(look, nsk)):
                    blocks[b] = qk_block(b)
                if denom == "dve":
                    ea = epool.tile([128, 512], BF16, tag="ea", bufs=2,
                                    name="ea")
                for b in range(nsk):
                    if b not in blocks:
                        blocks[b] = qk_block(b)
                    e, off, N = blocks.pop(b)
                    if denom == "dve":
                        if b == 0:
                            nc.vector.tensor_copy(ea[:], e[:])
                        else:
                            nc.vector.tensor_add(ea[:, off:512],
                                                 ea[:, off:512], e[:, 0:N])
                    nc.tensor.matmul(op[:, off:512],
                                     lhsT=v_sb[:, 128 * b: 128 * b + 128],
                                     rhs=e[:, 0:N],
                                     start=(b == 0), stop=(b == nsk - 1))
                    if denom == "pe":
                        nc.tensor.matmul(sm[0:1, off:512],
                                         lhsT=ones_sb[:, 0:1],
                                         rhs=e[:, 0:N],
                                         start=(b == 0), stop=(b == nsk - 1))
                    if b + look < nsk:
                        blocks[b + look] = qk_block(b + look)
                    blocks_done += 1
                    target = len(wo_q) * blocks_done // blocks_total
                    while emitted < target:
                        wo_q[emitted]()
                        emitted += 1

                # normalization chain: DVE + gpsimd only, PE rolls on
                rz = work.tile([1, 512], F32, tag="rz")
                if denom == "pe":
                    nc.vector.reciprocal(rz[:], sm[0:1, :])
                else:
                    zs = work.tile([128, 512], F32, tag="zs", bufs=2)
                    nc.gpsimd.partition_all_reduce(
                        zs[:], ea[:], channels=128,
                        reduce_op=bass_isa.ReduceOp.add)
                    nc.vector.reciprocal(rz[:], zs[0:1, :])
                rb = work.tile([1, 512], BF16, tag="rb")
                nc.vector.tensor_copy(rb[:], rz[:])
                bcs = work.tile([128, 512], BF16, tag="bcs", bufs=2)
                nc.gpsimd.partition_broadcast(bcs[:], rb[:])
                og = work.tile([128, 512], BF16, tag="og", bufs=8, name="og")
                nc.vector.tensor_mul(og[:], op[:], bcs[:])
                ogs.append(og)

            # drain any leftover wo work of the previous tile, then stage
            # this tile's wo for interleaving into the next tile's attention
            for em in wo_q[emitted:]:
                em()
            wo_work = wo_emitters(T, ogs, [(0, 4, 64 * T)])
            if T + 1 < ST:
                xt_ch = next_xt

        # last tile has no following attention to interleave with
        for em in wo_work:
            em()


# ---------------------------------------------------------------------------
# host-side preparation


def _prep_in_maps(x, wq, wk, wv, wo, freqs_cos, freqs_sin, mask):
    x2 = np.asarray(x, np.float32).reshape(S, D)
    perm = np.concatenate([np.arange(0, HD, 2), np.arange(1, HD, 2)])

    xT = np.ascontiguousarray(x2.T).astype(NPBF)                 # [D, S]
    # xtr[T][p, 512*b+f] = xT[128*b+p, 512*T+f]
    xtr = (xT.reshape(NB, 128, ST, 512).transpose(2, 1, 0, 3)
           .reshape(ST, 128, NB * 512))
    xtr = np.ascontiguousarray(xtr)

    # partition-doubled RoPE tables: [c;c] and [s;s]  (4-op RoPE)
    cst1 = np.asarray(freqs_cos, np.float32).T                   # [64, S]
    snt1 = np.asarray(freqs_sin, np.float32).T
    cstn = np.ascontiguousarray(np.concatenate([cst1, cst1], axis=0)).astype(NPBF)
    sntn = np.ascontiguousarray(np.concatenate([snt1, snt1], axis=0)).astype(NPBF)

    mnp = np.asarray(mask, np.float32)
    m01 = np.zeros((128, 4 * 512), np.float32)
    for r in range(4):
        m01[:, 512 * r: 512 * r + 512] = (mnp[0:512, 128 * r: 128 * r + 128].T == 0.0)
    m01 = m01.astype(NPBF)

    identity = np.eye(128, dtype=NPBF)

    wqn = np.asarray(wq, np.float32)
    wkn = np.asarray(wk, np.float32)
    wvn = np.asarray(wv, np.float32)
    won = np.asarray(wo, np.float32)

    in_maps = []
    for c in range(N_CORES):
        wq_c = wqn[:, c * QH * HD:(c + 1) * QH * HD].reshape(D, QH, HD)[:, :, perm]
        wq_c = wq_c.reshape(D, QH * HD)
        wk_c = wkn[:, c * HD:(c + 1) * HD][:, perm]
        wv_c = wvn[:, c * HD:(c + 1) * HD]
        wqkv_c = np.concatenate([wq_c, wk_c, wv_c], axis=1)      # [D, 768]
        wqkv_img = (wqkv_c.reshape(NB, 128, 768).transpose(1, 0, 2)
                    .reshape(128, NB * 768)).astype(NPBF)
        # wo ROW shard: woc[p, h*4096 + d] = wo[c*512 + h*128 + p, d]
        wob = won[c * 512:(c + 1) * 512, :]                      # [512, 4096]
        woc_img = (wob.reshape(QH, 128, D).transpose(1, 0, 2)
                   .reshape(128, QH * D)).astype(NPBF)
        in_maps.append({
            "xtr": xtr,
            "wqkv": np.ascontiguousarray(wqkv_img),
            "woc": np.ascontiguousarray(woc_img),
            "cst": cstn,
            "snt": sntn,
            "m01": np.ascontiguousarray(m01),
            "ident": identity,
        })
    return in_maps


def kernel(x, wq, wk, wv, wo, cache_k, cache_v, freqs_cos, freqs_sin, mask,
           input_indexes):
    """Full-input / full-output entry point.  cache_k/cache_v/input_indexes are
    consumed by the reference semantics (zero cache fully overwritten at
    positions arange(S)), so keys/values equal the fresh projections."""
    in_maps = _prep_in_maps(x, wq, wk, wv, wo, freqs_cos, freqs_sin, mask)
    if "nc" not in _CACHE:
        _CACHE["nc"] = build_kernel()
    nc = _CACHE["nc"]
    res = run_bass_kernel_spmd(nc, in_maps, core_ids=list(range(N_CORES)))
    full = np.empty((S, D), np.float32)
    for c in range(N_CORES):
        o = np.asarray(res.results[c]["out"]).astype(np.float32)  # [256, 4096]
        for T in range(ST):
            full[512 * T + 64 * c: 512 * T + 64 * (c + 1)] = o[64 * T: 64 * (T + 1)]
    return full.reshape(1, S, D)


if __name__ == "__main__":
    rng = np.random.default_rng(0)
    ins = {
        "x": rng.standard_normal((1, S, D), dtype=np.float32),
        "wq": (rng.standard_normal((D, H * HD), dtype=np.float32) * 0.02),
        "wk": (rng.standard_normal((D, KVH * HD), dtype=np.float32) * 0.02),
        "wv": (rng.standard_normal((D, KVH * HD), dtype=np.float32) * 0.02),
        "wo": (rng.standard_normal((H * HD, D), dtype=np.float32) * 0.02),
        "cache_k": np.zeros((1, S, KVH, HD), np.float32),
        "cache_v": np.zeros((1, S, KVH, HD), np.float32),
        "freqs_cos": rng.random((S, HD // 2), dtype=np.float32),
        "freqs_sin": rng.random((S, HD // 2), dtype=np.float32),
        "mask": np.triu(np.full((S, S), -1e9, dtype=np.float32), k=1),
        "input_indexes": np.arange(S, dtype=np.int32),
    }
    o = kernel(**ins)
    print("kernel output", o.shape, o.dtype)
